# revision 21
# baseline (speedup 1.0000x reference)
"""Column-sum kernel for Trainium2: out[d] = sum_r x[r, d].

x is [8192, 4096] f32, rows sharded across 8 NeuronCores (1024 rows
each). Per-core pipeline:

- Rows 0..767 load as six contiguous [128, 4096] row-tiles (2 MiB,
  fat descriptors -> full DMA rate) and fold into one [128, 4096]
  accumulator with an in-place DVE chain, hidden under the load
  stream.
- Rows 768..1023 load as four [128, 2, W] column-band blocks with
  tapering widths (last band smallest). Band c is the LAST data
  touching its columns, so as soon as it lands those columns fold
  (pair-add on GpSimd/DVE + acc add on DVE) and their ones-matmul
  partition reduce closes on the PE, copies to SBUF on ACT — all
  while later bands still stream. Output is written in two DMAs so
  the first 3 bands' columns fly out early.

The staggering kills the serial tail: a monolithic final [128, 4096]
reduce is ~9.5us of fp32 PE work (LOW_HIGH double pass) after the
last byte; here only the last small band's fold+close trails the
stream. Host sums the 8 per-core [1, 4096] partials.
"""

import numpy as np

M_CORES = 8
ROWS, D = 8192, 4096
ROWS_PER_CORE = ROWS // M_CORES  # 1024
P = 128
ROW_TILES = 6  # rows 0..767
BAND_J = 2  # rows 768..1023 as two 128-row sub-tiles per band
BAND_W = (1280, 1280, 1024, 512)  # tapering column bands, sum 4096
NCHUNK = 512  # fp32 PSUM bank capacity / max fp32 moving free dim

_nc_cache = None


def _build():
    import concourse.tile as tile
    from concourse import bacc, mybir

    nc = bacc.Bacc(None)
    x = nc.declare_dram_parameter(
        "x", [ROWS_PER_CORE, D], mybir.dt.float32, isOutput=False
    )
    out = nc.declare_dram_parameter("out", [1, D], mybir.dt.float32, isOutput=True)

    xband = x[ROW_TILES * P :, :].rearrange("(j p) d -> p j d", p=P)  # [128, 2, 4096]

    with tile.TileContext(nc) as tc:
        with (
            tc.tile_pool(name="xpool", bufs=4) as xpool,
            tc.tile_pool(name="bpool", bufs=4) as bpool,
            tc.tile_pool(name="vpool", bufs=2) as vpool,
            tc.tile_pool(name="singles", bufs=1) as singles,
            tc.tile_pool(name="psum", bufs=4, space="PSUM") as psum_pool,
        ):
            ones = singles.tile([P, 1], mybir.dt.float32)
            nc.vector.memset(ones[:], 1.0)

            osb = singles.tile([1, D], mybir.dt.float32)

            xts = []
            for k in range(ROW_TILES):
                xt = xpool.tile([P, D], mybir.dt.float32, name=f"xt{k}", tag="xt")
                nc.sync.dma_start(xt[:], x[k * P : (k + 1) * P, :])
                xts.append(xt)

            bts = []
            col = 0
            for c, W in enumerate(BAND_W):
                bt = bpool.tile([P, BAND_J * W], mybir.dt.float32,
                                name=f"bt{c}", tag="bt")
                nc.sync.dma_start(
                    bt[:].rearrange("p (j w) -> p j w", j=BAND_J),
                    xband[:, :, col : col + W],
                )
                bts.append(bt)
                col += W

            # Fold rows 0..767: in-place DVE chain, one add per arrival.
            acc = singles.tile([P, D], mybir.dt.float32)
            nc.vector.tensor_add(acc[:], xts[0][:], xts[1][:])
            for k in range(2, ROW_TILES):
                nc.vector.tensor_add(acc[:], acc[:], xts[k][:])

            # Per column band: pair-add the band sub-tiles (alternating
            # GpSimd/DVE so the post-chain DVE queue stays short), add the
            # accumulator slice on DVE, close the partition reduce on PE,
            # copy PSUM out on ACT.
            col = 0
            for c, W in enumerate(BAND_W):
                bt = bts[c]
                u = vpool.tile([P, W], mybir.dt.float32, name=f"u{c}", tag="u")
                eng = nc.gpsimd if c % 2 == 0 else nc.vector
                eng.tensor_add(u[:], bt[:, 0:W], bt[:, W : 2 * W])
                v = vpool.tile([P, W], mybir.dt.float32, name=f"v{c}", tag="v")
                nc.vector.tensor_add(v[:], u[:], acc[:, col : col + W])
                for s0 in range(0, W, NCHUNK):
                    sw = min(NCHUNK, W - s0)
                    ps = psum_pool.tile([1, NCHUNK], mybir.dt.float32,
                                        name=f"ps{c}_{s0}", tag="ps")
                    nc.tensor.matmul(
                        ps[:1, :sw], ones[:], v[:, s0 : s0 + sw],
                        start=True, stop=True,
                    )
                    nc.scalar.copy(osb[:, col + s0 : col + s0 + sw], ps[:1, :sw])
                col += W

            # First three bands' columns fly out as soon as their copies
            # land; only the last small band's columns trail the stream.
            split = sum(BAND_W[:3])
            nc.sync.dma_start(out[:, :split], osb[:, :split])
            nc.sync.dma_start(out[:, split:], osb[:, split:])

    nc.compile()
    return nc


def _get_nc():
    global _nc_cache
    if _nc_cache is None:
        _nc_cache = _build()
    return _nc_cache


def _run(x_np: np.ndarray, **run_kwargs):
    from concourse.bass_utils import run_bass_kernel_spmd

    nc = _get_nc()
    shards = np.split(x_np, M_CORES, axis=0)
    in_maps = [{"x": np.ascontiguousarray(s)} for s in shards]
    return run_bass_kernel_spmd(nc, in_maps, list(range(M_CORES)), **run_kwargs)


def kernel(x) -> np.ndarray:
    x_np = np.ascontiguousarray(np.asarray(x), dtype=np.float32)
    assert x_np.shape == (ROWS, D), x_np.shape
    res = _run(x_np)
    partials = np.stack([r["out"][0] for r in res.results])
    return partials.sum(axis=0, dtype=np.float32)
